# revision 10
# baseline (speedup 1.0000x reference)
"""CircleLoss on 8 Trainium2 NeuronCores — v3.

Math (reference):
    f = l2_normalize(features)              # (4096, 512)
    sim = f @ f.T                           # (4096, 4096), sim in [-1, 1]
    pos_term = -relu(1 + M - sim) * sim * G # M=0.25, G=256
    neg_term =  relu(sim + M) * sim * G
    loss = softplus(lse(pos_term | same-label) + lse(neg_term | diff-label))

Exact identities (sim <= 1 so relu(1.25-sim) is always active):
    pos_term = 256*(s-0.625)^2 - 100
    neg_term = 256*(relu(s+0.25)-0.125)^2 - 4

On-device masked forms (m_ij = [label_i == label_j], computed inline from
the column-label stream vs the per-row label scalar):
    qp = relu(m ? (0.625 - s) : 0)^2        -> pos exponent 256*qp - 100
    qn = relu(m ? 0 : (s + 0.125))^2        -> neg exponent 256*qn - 4
One custom DVE instruction per stream per 2048-wide chunk computes q AND
its running per-row max (accum=max).  Masked-out elements map to q=0,
far below the live per-row max, so they vanish in exp(256*(q - max)).
Approximation notes (relative impact < 1e-12 on the loss for unit-norm
random features): pos diagonal (s=1 -> relu(-0.375)=0; true term e^-64 vs
computed e^-100, both negligible vs lse_pos ~ +70); neg left clamp for
s < -0.125 (both ~e^-36 below the neg row max).

Matmuls run in bf16 (1 cyc/row on the PE vs 4 for fp32 LOW_HIGH), fp32
PSUM accumulation, k-outer so the stationary operand is reused across 4
consecutive matmuls.  Each core owns rows [c*512, (c+1)*512) of sim via
the roll-by-core trick (pure SPMD, static offsets).  exp/sum-exp runs
per (row-tile, 2048-group) against the group max; host combines group
partial sums exactly in float64 and applies the final softplus.
"""

import numpy as np
from contextlib import ExitStack

N = 4096
D = 512
NCORES = 8
ROWS_PER_CORE = N // NCORES          # 512
RT = ROWS_PER_CORE // 128            # 4 row-tiles per core
KT = D // 128                        # 4 k-tiles
GRP = 2048                           # custom-op span: 4 PSUM banks
NG = N // GRP                        # 2 groups per row-tile
SUB = GRP // 512                     # 512-wide matmul sub-chunks per group
NSTAT = RT * NG                      # 8 (t, g) stat slots per stream

_CACHE = {}

# Set by test.py to request a profiled run; kernel() stores the spmd result
# object here so the harness can read exec_time_ns / trace paths.
TRACE = False
LAST_RESULT = None


def _register_dve_ops():
    """Register the two fused CircleLoss DVE ops (idempotent).

    CIRCLE_POS2_ANT: out = relu(select(in1 == s0, imm2 - in0, 0))^2
    CIRCLE_NEG2_ANT: out = relu(select(in1 == s0, 0, in0 + imm2))^2
    both with accum_out = max(s1, max(out)).
    in1 = column-label stream, s0 = per-row label, s1 = running-max seed.
    """
    import concourse.dve_ops as dve_ops
    from concourse.dve_spec import (
        C0, C1, C2, Spec, Src0, Src1, Zero, lower, maxx, relu, sq, eq,
        select, _has_src1,
    )
    from concourse.dve_uop import DveOpSpec

    if "CIRCLE_POS2_ANT" in dve_ops._SUB_OPCODE_FOR_NAME:
        by_name = {op.name: op for op in dve_ops.OPS}
        return by_name["CIRCLE_POS2_ANT"], by_name["CIRCLE_NEG2_ANT"]

    def _pos_ref(in0, in1, s0, s1, imm2):
        x = in0.astype(np.float32).reshape(in0.shape[0], -1)
        lab = in1.astype(np.float32).reshape(x.shape)
        m = lab == np.asarray(s0, np.float32).reshape(-1, 1)
        body = np.maximum(np.where(m, np.float32(imm2) - x, 0.0), 0.0) ** 2
        acc = np.maximum(body.max(axis=-1, keepdims=True),
                         np.asarray(s1, np.float32).reshape(-1, 1))
        return body, acc

    def _neg_ref(in0, in1, s0, s1, imm2):
        x = in0.astype(np.float32).reshape(in0.shape[0], -1)
        lab = in1.astype(np.float32).reshape(x.shape)
        m = lab == np.asarray(s0, np.float32).reshape(-1, 1)
        body = np.maximum(np.where(m, 0.0, x + np.float32(imm2)), 0.0) ** 2
        acc = np.maximum(body.max(axis=-1, keepdims=True),
                         np.asarray(s1, np.float32).reshape(-1, 1))
        return body, acc

    specs = [
        ("CIRCLE_POS2_ANT",
         Spec(body=sq(relu(select(eq(Src1, C0), C2 - Src0, Zero))),
              accum=maxx, accum_init=C1, reference=_pos_ref)),
        ("CIRCLE_NEG2_ANT",
         Spec(body=sq(relu(select(eq(Src1, C0), Zero, Src0 + C2))),
              accum=maxx, accum_init=C1, reference=_neg_ref)),
    ]
    made = []
    for name, spec in specs:
        row = dve_ops._CUSTOM_DVE_ROW_BASE + len(dve_ops.OPS)
        assert row < 0x20
        dve_ops._SUB_OPCODE_FOR_NAME[name] = row
        shas = {}
        for ver in ("v3", "v4"):
            tmp = DveOpSpec(name=name, opcode=row, uops=lower(spec, ver=ver),
                            rd1_en=_has_src1(spec))
            shas[ver] = tmp.sha(ver)
        op = dve_ops.DveOp(name, spec, subdim=False, uops_sha=shas)
        dve_ops.OPS.append(op)
        dve_ops.CUSTOM_DVE_SPECS[name] = spec
        made.append(op)
    return made[0], made[1]


def _build_nc():
    import concourse.bacc as bacc
    import concourse.tile as tile
    from concourse import mybir

    POS_OP, NEG_OP = _register_dve_ops()

    f32 = mybir.dt.float32
    f16 = mybir.dt.float16
    bf16 = mybir.dt.bfloat16
    AF = mybir.ActivationFunctionType
    ALU = mybir.AluOpType

    nc = bacc.Bacc(None)
    # ft: [128, NB*KT*512] bf16 — host-interleaved so each 512-col block of
    # all 4 k-tiles is one contiguous 4KB-per-partition slab (identity DMA):
    # ft[p, b*2048 + k*512 + n] = fT[k*128 + p, b*512 + n]
    ft_h = nc.dram_tensor("ft", [128, KT * N], bf16, kind="ExternalInput")
    labbc_h = nc.dram_tensor("labbc", [128, N], f16, kind="ExternalInput")
    labloc_h = nc.dram_tensor("labloc", [128, RT], f32, kind="ExternalInput")
    stats_h = nc.dram_tensor("stats", [128, 4 * NSTAT], f32,
                             kind="ExternalOutput")

    NB = N // 512

    with tile.TileContext(nc) as tc, ExitStack() as ctx:
        persist = ctx.enter_context(tc.tile_pool(name="persist", bufs=1))
        qpool = ctx.enter_context(tc.tile_pool(name="qpool", bufs=2))
        sm = ctx.enter_context(tc.tile_pool(name="sm", bufs=4))
        ps = ctx.enter_context(tc.tile_pool(name="ps", bufs=2, space="PSUM"))

        ft_all = persist.tile([128, KT * N], bf16, tag="ft_all")
        labbc = persist.tile([128, N], f16, tag="labbc")
        labloc = persist.tile([128, RT], f32, tag="labloc")
        stats_t = persist.tile([128, 4 * NSTAT], f32, tag="stats")

        # Label DMAs ride the GpSimd queue, ft blocks the Sync queue, so the
        # trigger streams and transfers overlap and the first matmul can
        # start ~2.5us in instead of after the full 5MB.
        nc.gpsimd.dma_start(out=labloc[:], in_=labloc_h[:])
        for b in range(NB):
            sl = slice(b * 512, (b + 1) * 512)
            nc.gpsimd.dma_start(out=labbc[:, sl], in_=labbc_h[:, sl])
        for b in range(NB):
            sl = slice(b * 2048, (b + 1) * 2048)
            nc.sync.dma_start(out=ft_all[:, sl], in_=ft_h[:, sl])

        def w_ap(k, t):
            base = ((t * 128) // 512) * 2048 + k * 512 + (t * 128) % 512
            return ft_all[:, base:base + 128]

        def rhs_ap(k, bb):
            base = bb * 2048 + k * 512
            return ft_all[:, base:base + 512]

        for t in range(RT):
            posq = qpool.tile([128, N], f32, tag="posq")
            negq = qpool.tile([128, N], f32, tag="negq")

            for g in range(NG):
                pt = ps.tile([128, GRP], f32, tag="ps")
                for k in range(KT):
                    for sc in range(SUB):
                        out_sl = slice(sc * 512, (sc + 1) * 512)
                        mm = nc.tensor.matmul(
                            pt[:, out_sl], w_ap(k, t), rhs_ap(k, g * SUB + sc),
                            start=(k == 0), stop=(k == KT - 1),
                        )
                        if sc > 0:
                            # same stationary operand as the previous matmul:
                            # skip the redundant LDWEIGHTS
                            mm.ins.ldweights = False
                si = t * NG + g
                gsl = slice(g * GRP, (g + 1) * GRP)
                mp = stats_t[:, si:si + 1]
                mn = stats_t[:, NSTAT + si:NSTAT + si + 1]
                if t == 0 and g == 0:
                    # fine-grained first group: overlap with the tail of the
                    # input DMA stream (512-col customs, chained running max)
                    aps = ans = None
                    for sc in range(SUB):
                        ssl = slice(sc * 512, (sc + 1) * 512)
                        if sc < SUB - 1:
                            ap2 = sm.tile([128, 1], f32, tag=f"ap{sc}")
                            an2 = sm.tile([128, 1], f32, tag=f"an{sc}")
                        else:
                            ap2 = None
                            an2 = None
                        nc.vector._custom_dve(
                            POS_OP, out=posq[:, ssl], in0=pt[:, ssl],
                            in1=labbc[:, ssl], s0=labloc[:, t:t + 1],
                            s1=(0.0 if sc == 0 else aps[:]), imm2=0.625,
                            accum_out=(ap2[:] if ap2 is not None else mp),
                        )
                        nc.vector._custom_dve(
                            NEG_OP, out=negq[:, ssl], in0=pt[:, ssl],
                            in1=labbc[:, ssl], s0=labloc[:, t:t + 1],
                            s1=(0.0 if sc == 0 else ans[:]), imm2=0.125,
                            accum_out=(an2[:] if an2 is not None else mn),
                        )
                        aps, ans = ap2, an2
                else:
                    nc.vector._custom_dve(
                        POS_OP, out=posq[:, gsl], in0=pt[:],
                        in1=labbc[:, gsl], s0=labloc[:, t:t + 1],
                        s1=0.0, imm2=0.625, accum_out=mp,
                    )
                    nc.vector._custom_dve(
                        NEG_OP, out=negq[:, gsl], in0=pt[:],
                        in1=labbc[:, gsl], s0=labloc[:, t:t + 1],
                        s1=0.0, imm2=0.125, accum_out=mn,
                    )
                biasp = sm.tile([128, 1], f32, tag="biasp")
                biasn = sm.tile([128, 1], f32, tag="biasn")
                nc.gpsimd.tensor_scalar(biasp[:], mp, -256.0, None,
                                        op0=ALU.mult)
                nc.gpsimd.tensor_scalar(biasn[:], mn, -256.0, None,
                                        op0=ALU.mult)
                nc.scalar.activation(
                    posq[:, gsl], posq[:, gsl], AF.Exp, bias=biasp[:],
                    scale=256.0,
                    accum_out=stats_t[:, 2 * NSTAT + si:2 * NSTAT + si + 1])
                nc.scalar.activation(
                    negq[:, gsl], negq[:, gsl], AF.Exp, bias=biasn[:],
                    scale=256.0,
                    accum_out=stats_t[:, 3 * NSTAT + si:3 * NSTAT + si + 1])

        nc.sync.dma_start(out=stats_h[:], in_=stats_t[:])

    nc.finalize()
    return nc


def _get_nc():
    if "nc" not in _CACHE:
        _CACHE["nc"] = _build_nc()
    return _CACHE["nc"]


def _prep_inputs(features, labels):
    import ml_dtypes

    feats = np.asarray(features, dtype=np.float32)
    lab = np.asarray(labels).astype(np.float32)
    nrm = np.sqrt((feats.astype(np.float64) ** 2).sum(axis=1))
    nrm = np.maximum(nrm, 1e-12)
    f = (feats / nrm[:, None].astype(np.float32)).astype(np.float32)
    fT = np.ascontiguousarray(f.T)  # [D, N] fp32
    fT_bf = fT.astype(ml_dtypes.bfloat16)
    NB = N // 512
    in_maps = []
    for c in range(NCORES):
        sh = c * ROWS_PER_CORE
        lab_r = np.roll(lab, -sh)
        labloc = np.empty((128, RT), np.float32)
        for t in range(RT):
            labloc[:, t] = lab_r[t * 128:(t + 1) * 128]
        rolled = np.roll(fT_bf, -sh, axis=1)  # [D, N]
        # interleave: ft[p, b*2048 + k*512 + n] = rolled[k*128+p, b*512+n]
        ftp = rolled.reshape(KT, 128, NB, 512).transpose(1, 2, 0, 3)
        in_maps.append({
            "ft": np.ascontiguousarray(ftp.reshape(128, KT * N)),
            "labbc": np.ascontiguousarray(
                np.broadcast_to(lab_r.astype(np.float16), (128, N))),
            "labloc": labloc,
        })
    return in_maps


def _combine(stats_list):
    """Exact logsumexp combine from per-(row, group) (max-q, sumexp) stats."""
    mp, mn, sp, sn = [], [], [], []
    for st in stats_list:  # st: [128, 32]; col idx = t*NG + g
        def grab(base):
            # -> [512 rows, NG] ordered by global row within the core
            cols = st[:, base:base + NSTAT].reshape(128, RT, NG)
            return cols.transpose(1, 0, 2).reshape(ROWS_PER_CORE, NG)
        mp.append(grab(0))
        mn.append(grab(NSTAT))
        sp.append(grab(2 * NSTAT))
        sn.append(grab(3 * NSTAT))
    mp = np.concatenate(mp).astype(np.float64)
    mn = np.concatenate(mn).astype(np.float64)
    sp = np.concatenate(sp).astype(np.float64)
    sn = np.concatenate(sn).astype(np.float64)

    def row_lse(mg, sg, scale, off):
        # combine group partials: M = max_g, S = sum_g s_g*exp(scale*(m_g-M))
        M = mg.max(axis=1)
        S = (sg * np.exp(scale * (mg - M[:, None]))).sum(axis=1)
        Mt = scale * M + off
        g = Mt.max()
        return g + np.log((S * np.exp(Mt - g)).sum())

    lse_pos = row_lse(mp, sp, 256.0, -100.0)
    lse_neg = row_lse(mn, sn, 256.0, -4.0)
    loss = np.logaddexp(0.0, lse_pos + lse_neg)
    return np.asarray(loss, dtype=np.float32)


def kernel(features, labels):
    global LAST_RESULT
    from concourse.bass_utils import run_bass_kernel_spmd

    nc = _get_nc()
    in_maps = _prep_inputs(features, labels)
    res = run_bass_kernel_spmd(
        nc, in_maps, core_ids=list(range(NCORES)), trace=TRACE,
    )
    LAST_RESULT = res
    stats_list = [res.results[c]["stats"] for c in range(NCORES)]
    return _combine(stats_list)


# revision 13
# speedup vs baseline: 1.1330x; 1.1330x over previous
"""CircleLoss on 8 Trainium2 NeuronCores — v3.

Math (reference):
    f = l2_normalize(features)              # (4096, 512)
    sim = f @ f.T                           # (4096, 4096), sim in [-1, 1]
    pos_term = -relu(1 + M - sim) * sim * G # M=0.25, G=256
    neg_term =  relu(sim + M) * sim * G
    loss = softplus(lse(pos_term | same-label) + lse(neg_term | diff-label))

Exact identities (sim <= 1 so relu(1.25-sim) is always active):
    pos_term = 256*(s-0.625)^2 - 100
    neg_term = 256*(relu(s+0.25)-0.125)^2 - 4

On-device masked forms (m_ij = [label_i == label_j], computed inline from
the column-label stream vs the per-row label scalar):
    qp = relu(m ? (0.625 - s) : 0)^2        -> pos exponent 256*qp - 100
    qn = relu(m ? 0 : (s + 0.125))^2        -> neg exponent 256*qn - 4
One custom DVE instruction per stream per 2048-wide chunk computes q AND
its running per-row max (accum=max).  Masked-out elements map to q=0,
far below the live per-row max, so they vanish in exp(256*(q - max)).
Approximation notes (relative impact < 1e-12 on the loss for unit-norm
random features): pos diagonal (s=1 -> relu(-0.375)=0; true term e^-64 vs
computed e^-100, both negligible vs lse_pos ~ +70); neg left clamp for
s < -0.125 (both ~e^-36 below the neg row max).

Matmuls run in bf16 (1 cyc/row on the PE vs 4 for fp32 LOW_HIGH), fp32
PSUM accumulation, k-outer so the stationary operand is reused across 4
consecutive matmuls.  Each core owns rows [c*512, (c+1)*512) of sim via
the roll-by-core trick (pure SPMD, static offsets).  exp/sum-exp runs
per (row-tile, 2048-group) against the group max; host combines group
partial sums exactly in float64 and applies the final softplus.
"""

import numpy as np
from contextlib import ExitStack

N = 4096
D = 512
NCORES = 8
ROWS_PER_CORE = N // NCORES          # 512
RT = ROWS_PER_CORE // 128            # 4 row-tiles per core
KT = D // 128                        # 4 k-tiles
GRP = 2048                           # custom-op span: 4 PSUM banks
NG = N // GRP                        # 2 groups per row-tile
SUB = GRP // 512                     # 512-wide matmul sub-chunks per group
NSTAT = RT * NG                      # 8 (t, g) stat slots per stream

_CACHE = {}

# Set by test.py to request a profiled run; kernel() stores the spmd result
# object here so the harness can read exec_time_ns / trace paths.
TRACE = False
LAST_RESULT = None


def _register_dve_ops():
    """Register the two fused CircleLoss DVE ops (idempotent).

    CIRCLE_POS2_ANT: out = relu(select(in1 == s0, imm2 - in0, 0))^2
    CIRCLE_NEG2_ANT: out = relu(select(in1 == s0, 0, in0 + imm2))^2
    both with accum_out = max(s1, max(out)).
    in1 = column-label stream, s0 = per-row label, s1 = running-max seed.
    """
    import concourse.dve_ops as dve_ops
    from concourse.dve_spec import (
        C0, C1, C2, Spec, Src0, Src1, Zero, lower, maxx, relu, sq, eq,
        select, _has_src1,
    )
    from concourse.dve_uop import DveOpSpec

    if "CIRCLE_POS2_ANT" in dve_ops._SUB_OPCODE_FOR_NAME:
        by_name = {op.name: op for op in dve_ops.OPS}
        return by_name["CIRCLE_POS2_ANT"], by_name["CIRCLE_NEG2_ANT"]

    def _pos_ref(in0, in1, s0, s1, imm2):
        x = in0.astype(np.float32).reshape(in0.shape[0], -1)
        lab = in1.astype(np.float32).reshape(x.shape)
        m = lab == np.asarray(s0, np.float32).reshape(-1, 1)
        body = np.maximum(np.where(m, np.float32(imm2) - x, 0.0), 0.0) ** 2
        acc = np.maximum(body.max(axis=-1, keepdims=True),
                         np.asarray(s1, np.float32).reshape(-1, 1))
        return body, acc

    def _neg_ref(in0, in1, s0, s1, imm2):
        x = in0.astype(np.float32).reshape(in0.shape[0], -1)
        lab = in1.astype(np.float32).reshape(x.shape)
        m = lab == np.asarray(s0, np.float32).reshape(-1, 1)
        body = np.maximum(np.where(m, 0.0, x + np.float32(imm2)), 0.0) ** 2
        acc = np.maximum(body.max(axis=-1, keepdims=True),
                         np.asarray(s1, np.float32).reshape(-1, 1))
        return body, acc

    specs = [
        ("CIRCLE_POS2_ANT",
         Spec(body=sq(relu(select(eq(Src1, C0), C2 - Src0, Zero))),
              accum=maxx, accum_init=C1, reference=_pos_ref)),
        ("CIRCLE_NEG2_ANT",
         Spec(body=sq(relu(select(eq(Src1, C0), Zero, Src0 + C2))),
              accum=maxx, accum_init=C1, reference=_neg_ref)),
    ]
    made = []
    for name, spec in specs:
        row = dve_ops._CUSTOM_DVE_ROW_BASE + len(dve_ops.OPS)
        assert row < 0x20
        dve_ops._SUB_OPCODE_FOR_NAME[name] = row
        shas = {}
        for ver in ("v3", "v4"):
            tmp = DveOpSpec(name=name, opcode=row, uops=lower(spec, ver=ver),
                            rd1_en=_has_src1(spec))
            shas[ver] = tmp.sha(ver)
        op = dve_ops.DveOp(name, spec, subdim=False, uops_sha=shas)
        dve_ops.OPS.append(op)
        dve_ops.CUSTOM_DVE_SPECS[name] = spec
        made.append(op)
    return made[0], made[1]


def _build_nc():
    import concourse.bacc as bacc
    import concourse.tile as tile
    from concourse import mybir

    POS_OP, NEG_OP = _register_dve_ops()

    f32 = mybir.dt.float32
    f16 = mybir.dt.float16
    bf16 = mybir.dt.bfloat16
    AF = mybir.ActivationFunctionType
    ALU = mybir.AluOpType

    nc = bacc.Bacc(None)
    # ft: [KT*128, N] bf16 (transposed normalized features, k-tiles stacked)
    ft_h = nc.dram_tensor("ft", [D, N], bf16, kind="ExternalInput")
    labbc_h = nc.dram_tensor("labbc", [128, N], f16, kind="ExternalInput")
    labloc_h = nc.dram_tensor("labloc", [128, RT], f32, kind="ExternalInput")
    stats_h = nc.dram_tensor("stats", [128, 4 * NSTAT], f32,
                             kind="ExternalOutput")

    ft_v = ft_h[:].rearrange("(kt p) n -> kt p n", p=128)   # [KT, 128, N]

    with tile.TileContext(nc) as tc, ExitStack() as ctx:
        persist = ctx.enter_context(tc.tile_pool(name="persist", bufs=1))
        qpool = ctx.enter_context(tc.tile_pool(name="qpool", bufs=2))
        sm = ctx.enter_context(tc.tile_pool(name="sm", bufs=4))
        ps = ctx.enter_context(tc.tile_pool(name="ps", bufs=2, space="PSUM"))

        # ft_all holds the 4 k-tiles side by side: free index = k*N + col.
        ft_all = persist.tile([128, KT * N], bf16, tag="ft_all")
        labbc = persist.tile([128, N], f16, tag="labbc")
        labloc = persist.tile([128, RT], f32, tag="labloc")
        stats_t = persist.tile([128, 4 * NSTAT], f32, tag="stats")

        nc.sync.dma_start(out=labloc[:], in_=labloc_h[:])
        # Column-block streaming: per 512-col block, one DMA carrying all 4
        # k-tiles ([KT,128,512] -> [128,KT,512]) plus the labels block, so
        # the first matmul can start early instead of after the full 4MB.
        ftv3 = ft_all[:].rearrange("p (kt n) -> p kt n", kt=KT)
        NB = N // 512
        for b in range(NB):
            sl = slice(b * 512, (b + 1) * 512)
            nc.sync.dma_start(out=labbc[:, sl], in_=labbc_h[:, sl])
            nc.sync.dma_start(out=ftv3[:, :, sl], in_=ft_v[:, :, sl])

        def w_ap(k, t):
            base = k * N + t * 128
            return ft_all[:, base:base + 128]

        def rhs_ap(k, bb):
            base = k * N + bb * 512
            return ft_all[:, base:base + 512]

        for t in range(RT):
            posq = qpool.tile([128, N], f32, tag="posq")
            negq = qpool.tile([128, N], f32, tag="negq")

            for g in range(NG):
                pt = ps.tile([128, GRP], f32, tag="ps")
                for k in range(KT):
                    for sc in range(SUB):
                        out_sl = slice(sc * 512, (sc + 1) * 512)
                        nc.tensor.matmul(
                            pt[:, out_sl], w_ap(k, t), rhs_ap(k, g * SUB + sc),
                            start=(k == 0), stop=(k == KT - 1),
                        )
                si = t * NG + g
                gsl = slice(g * GRP, (g + 1) * GRP)
                mp = stats_t[:, si:si + 1]
                mn = stats_t[:, NSTAT + si:NSTAT + si + 1]
                if t == 0 and g == 0:
                    # fine-grained first group: overlap with the tail of the
                    # input DMA stream (512-col customs, chained running max)
                    aps = ans = None
                    for sc in range(SUB):
                        ssl = slice(sc * 512, (sc + 1) * 512)
                        if sc < SUB - 1:
                            ap2 = sm.tile([128, 1], f32, tag=f"ap{sc}")
                            an2 = sm.tile([128, 1], f32, tag=f"an{sc}")
                        else:
                            ap2 = None
                            an2 = None
                        nc.vector._custom_dve(
                            POS_OP, out=posq[:, ssl], in0=pt[:, ssl],
                            in1=labbc[:, ssl], s0=labloc[:, t:t + 1],
                            s1=(0.0 if sc == 0 else aps[:]), imm2=0.625,
                            accum_out=(ap2[:] if ap2 is not None else mp),
                        )
                        nc.vector._custom_dve(
                            NEG_OP, out=negq[:, ssl], in0=pt[:, ssl],
                            in1=labbc[:, ssl], s0=labloc[:, t:t + 1],
                            s1=(0.0 if sc == 0 else ans[:]), imm2=0.125,
                            accum_out=(an2[:] if an2 is not None else mn),
                        )
                        aps, ans = ap2, an2
                else:
                    nc.vector._custom_dve(
                        POS_OP, out=posq[:, gsl], in0=pt[:],
                        in1=labbc[:, gsl], s0=labloc[:, t:t + 1],
                        s1=0.0, imm2=0.625, accum_out=mp,
                    )
                    nc.vector._custom_dve(
                        NEG_OP, out=negq[:, gsl], in0=pt[:],
                        in1=labbc[:, gsl], s0=labloc[:, t:t + 1],
                        s1=0.0, imm2=0.125, accum_out=mn,
                    )
                biasp = sm.tile([128, 1], f32, tag="biasp")
                biasn = sm.tile([128, 1], f32, tag="biasn")
                nc.gpsimd.tensor_scalar(biasp[:], mp, -256.0, None,
                                        op0=ALU.mult)
                nc.gpsimd.tensor_scalar(biasn[:], mn, -256.0, None,
                                        op0=ALU.mult)
                nc.scalar.activation(
                    posq[:, gsl], posq[:, gsl], AF.Exp, bias=biasp[:],
                    scale=256.0,
                    accum_out=stats_t[:, 2 * NSTAT + si:2 * NSTAT + si + 1])
                nc.scalar.activation(
                    negq[:, gsl], negq[:, gsl], AF.Exp, bias=biasn[:],
                    scale=256.0,
                    accum_out=stats_t[:, 3 * NSTAT + si:3 * NSTAT + si + 1])

        nc.sync.dma_start(out=stats_h[:], in_=stats_t[:])

    nc.finalize()
    return nc


def _get_nc():
    if "nc" not in _CACHE:
        _CACHE["nc"] = _build_nc()
    return _CACHE["nc"]


def _prep_inputs(features, labels):
    import ml_dtypes

    feats = np.asarray(features, dtype=np.float32)
    lab = np.asarray(labels).astype(np.float32)
    nrm = np.sqrt((feats.astype(np.float64) ** 2).sum(axis=1))
    nrm = np.maximum(nrm, 1e-12)
    f = (feats / nrm[:, None].astype(np.float32)).astype(np.float32)
    fT = np.ascontiguousarray(f.T)  # [D, N] fp32
    fT_bf = fT.astype(ml_dtypes.bfloat16)
    in_maps = []
    for c in range(NCORES):
        sh = c * ROWS_PER_CORE
        lab_r = np.roll(lab, -sh)
        labloc = np.empty((128, RT), np.float32)
        for t in range(RT):
            labloc[:, t] = lab_r[t * 128:(t + 1) * 128]
        in_maps.append({
            "ft": np.ascontiguousarray(np.roll(fT_bf, -sh, axis=1)),
            "labbc": np.ascontiguousarray(
                np.broadcast_to(lab_r.astype(np.float16), (128, N))),
            "labloc": labloc,
        })
    return in_maps


def _combine(stats_list):
    """Exact logsumexp combine from per-(row, group) (max-q, sumexp) stats."""
    mp, mn, sp, sn = [], [], [], []
    for st in stats_list:  # st: [128, 32]; col idx = t*NG + g
        def grab(base):
            # -> [512 rows, NG] ordered by global row within the core
            cols = st[:, base:base + NSTAT].reshape(128, RT, NG)
            return cols.transpose(1, 0, 2).reshape(ROWS_PER_CORE, NG)
        mp.append(grab(0))
        mn.append(grab(NSTAT))
        sp.append(grab(2 * NSTAT))
        sn.append(grab(3 * NSTAT))
    mp = np.concatenate(mp).astype(np.float64)
    mn = np.concatenate(mn).astype(np.float64)
    sp = np.concatenate(sp).astype(np.float64)
    sn = np.concatenate(sn).astype(np.float64)

    def row_lse(mg, sg, scale, off):
        # combine group partials: M = max_g, S = sum_g s_g*exp(scale*(m_g-M))
        M = mg.max(axis=1)
        S = (sg * np.exp(scale * (mg - M[:, None]))).sum(axis=1)
        Mt = scale * M + off
        g = Mt.max()
        return g + np.log((S * np.exp(Mt - g)).sum())

    lse_pos = row_lse(mp, sp, 256.0, -100.0)
    lse_neg = row_lse(mn, sn, 256.0, -4.0)
    loss = np.logaddexp(0.0, lse_pos + lse_neg)
    return np.asarray(loss, dtype=np.float32)


def kernel(features, labels):
    global LAST_RESULT
    from concourse.bass_utils import run_bass_kernel_spmd

    nc = _get_nc()
    in_maps = _prep_inputs(features, labels)
    res = run_bass_kernel_spmd(
        nc, in_maps, core_ids=list(range(NCORES)), trace=TRACE,
    )
    LAST_RESULT = res
    stats_list = [res.results[c]["stats"] for c in range(NCORES)]
    return _combine(stats_list)
